# revision 54
# baseline (speedup 1.0000x reference)
"""CronRootAttention (causal sqrt-N sparse attention + GQA projections) on 8 TRN2 cores.

Sharding: pure sequence shard — each core owns 256 queries, computes all 16
heads for them. Weights replicated; kv projections computed per-core for the
local 320-key span plus the 44 shared strided keys.

v4 dataflow:
  - Local attention in 64-query blocks: each block's 46-wide causal band fits
    one [128-key, 64-query] tile. Blocks are packed in pairs so every phase-B
    PSUM tile is exactly one bank ([128,512] f32) and every mask is an
    additive PE identity-preload (exp then reads masked scores directly).
  - q-projection runs as 4 mini-passes (2 accumulators each) interleaved
    INTO the attention g-loop: mini-pass g+1's matmuls sit between g's score
    and PV sections, covering the exp/normalize engine latencies, and each
    mini-pass is paced by one contiguous wq quarter-chunk DMA.
  - PV head-pairs accumulate into one [65,512] bank; denominators (ones
    column of v) are normalized via one batched reciprocal + gpsimd
    partition-broadcast + two DVE muls per pair.
  - Output projection is one solid PE streak at the end (k-tile-major, so
    the last g's normalize latency hides under k-tiles 0-5), staged out as
    two [128,1024] DMAs.
  - Inputs packed into 5 dram tensors in k-tile-interleaved layout, DMA'd in
    PE-consumption order (transfers are a serial ~332B/ns stream).
"""

import math
import sys

sys.path.insert(0, "/opt/trn_rl_repo")

import numpy as np
import ml_dtypes
import concourse.bass as bass
import concourse.tile as tile
from concourse import bacc, mybir
from concourse.bass_utils import run_bass_kernel_spmd

F32 = mybir.dt.float32
BF16 = mybir.dt.bfloat16
EXP = mybir.ActivationFunctionType.Exp
COPY = mybir.ActivationFunctionType.Copy
MULT = mybir.AluOpType.mult

MM_DT = BF16
NP_DT = ml_dtypes.bfloat16

# Problem constants (hardcoded per contract).
B, S, D = 1, 2048, 1024
H, H_KV, HD = 16, 4, 64
W = int(math.ceil(math.sqrt(S)))  # 46
NCORES = 8
SQ = S // NCORES  # 256 queries per core
KSPAN = 320  # local key span: [qs-64, qs+256)
SIDX = np.arange(W - 1, S, W)  # strided key positions
NS = len(SIDX)  # 44
KT = D // 128  # 8 contraction k-tiles
MASKV = -400.0  # additive mask value; exp(0.125*(qk-400)) == 0

# packed t_a column layout per k-tile: [xkv 320 | wk 256 | wv 260]
TA_W = KSPAN + 256 + 260  # 836
# packed aux layout: [xs 8*44 | ident 128 | mquad 4*256]
AUX_XS, AUX_ID, AUX_MQ = 0, 352, 480
AUX_W = 480 + 1024  # 1504


def build_nc():
    nc = bacc.Bacc("TRN2", target_bir_lowering=False, debug=False, num_devices=1)
    ta = nc.dram_tensor("ta", [128, KT, TA_W], MM_DT, kind="ExternalInput").ap()
    # wq chunk-major: [128, 4 col-chunks, KT, 256]; one DMA per mini-pass.
    wq = nc.dram_tensor("wq", [128, 4, KT, 256], MM_DT, kind="ExternalInput").ap()
    wo = nc.dram_tensor("wo", [128, KT, D], MM_DT, kind="ExternalInput").ap()
    aux = nc.dram_tensor("aux", [128, AUX_W], MM_DT, kind="ExternalInput").ap()
    msadd = nc.dram_tensor("msadd", [NS, 1024], MM_DT, kind="ExternalInput").ap()
    y = nc.dram_tensor("y", [SQ, D], MM_DT, kind="ExternalOutput").ap()

    with tile.TileContext(nc) as tc:
        with (
            tc.tile_pool(name="consts", bufs=1) as consts,
            tc.tile_pool(name="work", bufs=1) as work,
        ):
            ta_sb = consts.tile([128, KT, TA_W], MM_DT)
            wq_sb = consts.tile([128, 4, KT, 256], MM_DT)
            wo_sb = consts.tile([128, KT, D], MM_DT)
            aux_sb = consts.tile([128, AUX_W], MM_DT)
            msadd_sb = consts.tile([NS, 1024], MM_DT)

            # DMA stream in PE-consumption order; aux lands before ta[4:8]
            # so the ks/vs matmuls can fill the PE hole while ta[4:8] is in
            # flight.
            nc.sync.dma_start(out=ta_sb[:, 0, 0:576], in_=ta[:, 0, 0:576])
            nc.sync.dma_start(out=ta_sb[:, 0, 576:836], in_=ta[:, 0, 576:836])
            nc.sync.dma_start(out=ta_sb[:, 1:2, :], in_=ta[:, 1:2])
            nc.sync.dma_start(out=ta_sb[:, 2:4, :], in_=ta[:, 2:4])
            nc.sync.dma_start(out=ta_sb[:, 4:6, :], in_=ta[:, 4:6])
            nc.sync.dma_start(out=ta_sb[:, 6:8, :], in_=ta[:, 6:8])
            nc.sync.dma_start(out=aux_sb[:], in_=aux)
            nc.sync.dma_start(out=msadd_sb[:], in_=msadd)
            for c in range(4):
                nc.sync.dma_start(out=wq_sb[:, c], in_=wq[:, c])
            for t in range(2):
                nc.sync.dma_start(out=wo_sb[:, 4 * t : 4 * t + 4, :],
                                  in_=wo[:, 4 * t : 4 * t + 4])

            def xkv(kt):
                return ta_sb[:, kt, 0:KSPAN]

            def wk_sl(kt, ot):
                return ta_sb[:, kt, KSPAN + 128 * ot : KSPAN + 128 * ot + 128]

            def wv_sl(kt):
                return ta_sb[:, kt, KSPAN + 256 : KSPAN + 256 + 260]

            xs_v = aux_sb[:, AUX_XS : AUX_XS + KT * NS].rearrange(
                "p (kt s) -> p kt s", kt=KT
            )
            id_sb = aux_sb[:, AUX_ID : AUX_ID + 128]
            mquad = aux_sb[:, AUX_MQ : AUX_MQ + 1024]

            # work tiles (attention matmul operands all at base partition 0)
            q_sb = work.tile([64, H, SQ], MM_DT)
            k_sb = work.tile([64, 4, KSPAN], MM_DT)
            ks_sb = work.tile([64, 4, NS], MM_DT)
            v_sb = work.tile([128, 4, 260], MM_DT)  # 4 stationary alignments
            vs_sb = work.tile([NS, 260], MM_DT)
            attn_sb = work.tile([128, KT, SQ], MM_DT)

            _eng = [0]

            def copy_any(out, in_):
                e = _eng[0] % 2
                _eng[0] += 1
                if e == 0:
                    nc.scalar.activation(out, in_, COPY)
                else:
                    nc.vector.tensor_copy(out, in_)

            # ---- phase A: k, v, ks, vs projections ----
            with tc.tile_pool(name="psA2a", bufs=1, space="PSUM") as psA2a:
                with tc.tile_pool(name="psA1", bufs=1, space="PSUM") as psA1:
                    kps = [psA1.tile([128, 512], F32, tag="kp", bufs=2, name=f"kp{_}") for _ in range(2)]
                    vps = [psA1.tile([128, 512], F32, tag="vp", bufs=3, name=f"vp{_}") for _ in range(3)]
                    vsp = psA2a.tile([128, 512], F32, tag="vsp")
                    ksps = [psA2a.tile([128, 64], F32, tag="ksp", bufs=2, name=f"ksp{_}") for _ in range(2)]

                    def wave1_kt(kt):
                        st, sp = kt == 0, kt == KT - 1
                        for ot in range(2):
                            nc.tensor.matmul(
                                kps[ot][:, 0:KSPAN], wk_sl(kt, ot), xkv(kt),
                                start=st, stop=sp,
                            )
                        for mt in range(2):
                            nc.tensor.matmul(
                                vps[mt][:, 0:260],
                                ta_sb[:, kt, 128 * mt : 128 * mt + 128],
                                wv_sl(kt),
                                start=st, stop=sp,
                            )
                        nc.tensor.matmul(
                            vps[2][0:64, 0:260], ta_sb[:, kt, 256:320], wv_sl(kt),
                            start=st, stop=sp,
                        )

                    def ksvs_kt(kt):
                        st, sp = kt == 0, kt == KT - 1
                        for ot in range(2):
                            nc.tensor.matmul(
                                ksps[ot][:, 0:NS], wk_sl(kt, ot), xs_v[:, kt, :],
                                start=st, stop=sp,
                            )
                        nc.tensor.matmul(
                            vsp[0:NS, 0:260], xs_v[:, kt, :], wv_sl(kt),
                            start=st, stop=sp,
                        )

                    for kt in range(KT):
                        wave1_kt(kt)
                    for kt in range(KT):
                        ksvs_kt(kt)
                    for ot in range(2):
                        copy_any(k_sb[:, 2 * ot, :], kps[ot][0:64, 0:KSPAN])
                        copy_any(k_sb[:, 2 * ot + 1, :], kps[ot][64:128, 0:KSPAN])
                    # v alignments: tiles cover span rows [64b, 64b+128)
                    copy_any(v_sb[:, 0, :], vps[0][:, 0:260])
                    copy_any(v_sb[:, 2, :], vps[1][:, 0:260])
                    copy_any(v_sb[0:64, 1, :], vps[0][64:128, 0:260])
                    copy_any(v_sb[64:128, 1, :], vps[1][0:64, 0:260])
                    copy_any(v_sb[0:64, 3, :], vps[1][64:128, 0:260])
                    copy_any(v_sb[64:128, 3, :], vps[2][0:64, 0:260])
                    for t in range(4):
                        ones_cols = v_sb[:, t, :].rearrange(
                            "p (g c) -> p g c", g=4
                        )[:, :, 64]
                        nc.gpsimd.memset(ones_cols, 1.0)
                for ot in range(2):
                    copy_any(ks_sb[:, 2 * ot, :], ksps[ot][0:64, 0:NS])
                    copy_any(ks_sb[:, 2 * ot + 1, :], ksps[ot][64:128, 0:NS])
                copy_any(vs_sb[:], vsp[0:NS, 0:260])
                vs_ones = vs_sb[:].rearrange("p (g c) -> p g c", g=4)[:, :, 64]
                nc.gpsimd.memset(vs_ones, 1.0)

            # ---- phase B: attention g-loop with q mini-passes woven in ----
            def q_sl(h, c0, c1):
                return q_sb[:, h, c0:c1]

            def k_sl(g, b):
                return k_sb[:, g, 64 * b : 64 * b + 128]

            with (
                tc.tile_pool(name="ps_blk", bufs=1, space="PSUM") as psb,
                tc.tile_pool(name="ps_pv", bufs=1, space="PSUM") as pspv,
                tc.tile_pool(name="ptiles", bufs=1) as pt,
                tc.tile_pool(name="small", bufs=1) as sm,
            ):
                def qpass(c):
                    qps = [
                        psq.tile([128, SQ], F32, tag="qp", bufs=2,
                                 name=f"qp{c}{_}")
                        for _ in range(2)
                    ]
                    for kt in range(KT):
                        st, sp = kt == 0, kt == KT - 1
                        for i in range(2):
                            nc.tensor.matmul(
                                qps[i][:],
                                wq_sb[:, c, kt, 128 * i : 128 * i + 128],
                                xkv(kt)[:, 64:320],
                                start=st, stop=sp,
                            )
                    for i in range(2):
                        ot = 2 * c + i
                        copy_any(q_sb[:, 2 * ot, :], qps[i][0:64, :])
                        copy_any(q_sb[:, 2 * ot + 1, :], qps[i][64:128, :])

                def scores_g(g):
                    # strided: 2 head-pair tiles [44,512]; mask via identity
                    # preload, so exp output is final. Exp results land in
                    # the per-g halves of merged pstr/pbq SBUF tiles.
                    pstr = pt.tile([NS, 1024], MM_DT, tag="pstr", bufs=3)
                    pbq = pt.tile([128, 1024], MM_DT, tag="pb", bufs=3)
                    for u in range(2):
                        stt = psb.tile([NS, 512], F32, tag="blk", bufs=4,
                                       name=f"st{u}")
                        nc.tensor.matmul(
                            stt[:], id_sb[0:NS, 0:NS],
                            msadd_sb[:, 512 * u : 512 * u + 512],
                            start=True, stop=False, skip_group_check=True,
                        )
                        for i in range(2):
                            nc.tensor.matmul(
                                stt[:, 256 * i : 256 * i + 256],
                                ks_sb[:, g, :],
                                q_sl(4 * g + 2 * u + i, 0, SQ),
                                start=False, stop=True,
                                skip_group_check=True,
                            )
                        nc.scalar.activation(
                            pstr[:, 512 * u : 512 * u + 512], stt[:], EXP,
                            scale=0.125,
                        )
                    # local: block pairs packed into one bank [128, 512]
                    for half in range(2):
                        spp = psb.tile([128, 512], F32, tag="blk", bufs=4,
                                       name=f"sp{half}")
                        nc.tensor.matmul(
                            spp[:], id_sb,
                            mquad[:, 512 * half : 512 * half + 512],
                            start=True, stop=False, skip_group_check=True,
                        )
                        for j in range(2):
                            b = 2 * half + j
                            for hh in range(4):
                                nc.tensor.matmul(
                                    spp[:, 256 * j + 64 * hh : 256 * j + 64 * hh + 64],
                                    k_sl(g, b),
                                    q_sl(4 * g + hh, 64 * b, 64 * b + 64),
                                    start=False, stop=(j == 1 and hh == 3),
                                    skip_group_check=True,
                                )
                        nc.scalar.activation(
                            pbq[:, 512 * half : 512 * half + 512], spp[:], EXP,
                            scale=0.125,
                        )
                    return pstr, pbq

                def pv_g(g, pstr, pbq):
                    # PV per head pair into one [65,512] bank + normalize.
                    # g3 runs pair 1 (k-tile 7) first so phase C's last
                    # k-tiles unblock in emission order.
                    for u in (0, 1) if g < 3 else (1, 0):
                        pvp = pspv.tile([65, 512], F32, tag="pv", bufs=2)
                        for i in range(2):
                            hh = 2 * u + i
                            c = 256 * i
                            nc.tensor.matmul(
                                pvp[:, c : c + 256],
                                vs_sb[:, 65 * g : 65 * g + 65],
                                pstr[:, 256 * hh : 256 * hh + 256],
                                start=True, stop=False,
                                skip_group_check=True,
                            )
                            for b in range(4):
                                nc.tensor.matmul(
                                    pvp[:, c + 64 * b : c + 64 * b + 64],
                                    v_sb[:, b, 65 * g : 65 * g + 65],
                                    pbq[:, 256 * b + 64 * hh : 256 * b + 64 * hh + 64],
                                    start=False, stop=b == 3,
                                    skip_group_check=True,
                                )
                        rt = sm.tile([1, 512], F32, tag="rt", bufs=3)
                        nc.vector.reciprocal(rt[:], pvp[64:65, :])
                        rep = sm.tile([64, 512], F32, tag="rep", bufs=3)
                        nc.gpsimd.partition_broadcast(rep[:], rt[:], channels=64)
                        for i in range(2):
                            h = 4 * g + 2 * u + i
                            nc.vector.tensor_tensor(
                                out=attn_sb[64 * (h % 2) : 64 * (h % 2) + 64, h // 2, :],
                                in0=pvp[0:64, 256 * i : 256 * i + 256],
                                in1=rep[:, 256 * i : 256 * i + 256],
                                op=MULT,
                            )

                with tc.tile_pool(name="psq", bufs=1, space="PSUM") as psq:
                    qpass(0)
                    for g in range(4):
                        sc = scores_g(g)
                        if g < 3:
                            qpass(g + 1)
                        pv_g(g, *sc)

                # ---- phase C: output projection. The 4 y accumulators take
                # the blk ring's 4 banks (no pool-close barrier separates
                # it from phase B); k-tile-major so the last g's normalize
                # latency hides under k-tiles 0-5.
                with tc.tile_pool(name="yout", bufs=2) as yo:
                    CHUNKS = [(0, 0), (0, 512), (1, 0), (1, 512)]
                    yts = [psb.tile([128, 512], F32, tag="blk", bufs=4,
                                    name=f"yt{i}") for i in range(4)]
                    for kt in range(KT):
                        last = kt == KT - 1
                        # qt1 chunks first on the last k-tile so their
                        # staging starts earlier.
                        order = (2, 3, 0, 1) if last else (0, 1, 2, 3)
                        for i in order:
                            qt, c0 = CHUNKS[i]
                            nc.tensor.matmul(
                                yts[i][:],
                                attn_sb[:, kt, 128 * qt : 128 * qt + 128],
                                wo_sb[:, kt, c0 : c0 + 512],
                                start=kt == 0,
                                stop=last,
                            )
                    for qt in (1, 0):
                        ysb = yo.tile([128, D], MM_DT, tag="ysb", name=f"ysb{qt}")
                        nc.scalar.activation(ysb[:, 0:512], yts[2 * qt][:], COPY)
                        nc.vector.tensor_copy(ysb[:, 512:1024], yts[2 * qt + 1][:])
                        nc.sync.dma_start(
                            out=y[128 * qt : 128 * qt + 128, :], in_=ysb[:]
                        )
    nc.compile()
    return nc


def host_prep(x, Wq, Wk, Wv, Wo):
    """Build per-core input maps (pure data reordering, no FLOPs)."""
    x2 = np.asarray(x, np.float32).reshape(S, D)
    xT = np.ascontiguousarray(x2.T)  # [D, S]
    xpad = np.zeros((D, 64 + S), np.float32)
    xpad[:, 64:] = xT
    xs = xT[:, SIDX]  # [D, 44]
    wkT = np.asarray(Wk, np.float32).T  # [D, 256]
    wvT = np.asarray(Wv, np.float32).T  # [D, 256]
    wv = np.zeros((D, 260), np.float32)
    for g in range(4):
        wv[:, 65 * g : 65 * g + 64] = wvT[:, 64 * g : 64 * g + 64]
    # wq chunk-major: [128, 4, KT, 256]; chunk c = output channels 256c..
    wq_t = np.ascontiguousarray(
        np.asarray(Wq, np.float32).T.reshape(KT, 128, 4, 256).transpose(1, 2, 0, 3)
    ).astype(NP_DT)
    wo_t = np.ascontiguousarray(
        np.asarray(Wo, np.float32).T.reshape(KT, 128, D).transpose(1, 0, 2)
    ).astype(NP_DT)

    # local band mask: i = qb + c, j = qb - 64 + r -> valid iff
    # 0 <= c + 64 - r <= 45 (identical for every block except block 0 of
    # core 0, whose keys j<0 must also be killed).
    r = np.arange(128)[:, None]
    c = np.arange(64)[None, :]
    band = (c + 64 - r >= 0) & (c + 64 - r <= 45)
    mloc1 = np.tile(np.where(band, 0.0, MASKV).astype(np.float32), (1, 4))

    ident = np.eye(128, dtype=np.float32)

    in_maps = []
    for core in range(NCORES):
        qs = SQ * core
        xkv = xpad[:, qs : qs + KSPAN]  # [D, 320]
        ta = np.concatenate(
            [
                xkv.reshape(KT, 128, KSPAN),
                wkT.reshape(KT, 128, 256),
                wv.reshape(KT, 128, 260),
            ],
            axis=2,
        ).transpose(1, 0, 2)
        ta = np.ascontiguousarray(ta).astype(NP_DT)
        jglob = qs - 64 + np.arange(128)[:, None]
        band0 = band & (jglob >= 0)
        mloc0 = np.tile(np.where(band0, 0.0, MASKV).astype(np.float32), (1, 4))
        auxa = np.zeros((128, AUX_W), np.float32)
        auxa[:, AUX_XS : AUX_XS + KT * NS] = (
            xs.reshape(KT, 128, NS).transpose(1, 0, 2).reshape(128, KT * NS)
        )
        auxa[:, AUX_ID : AUX_ID + 128] = ident
        # mquad: [block0 mask | block1-3 mask x3]
        auxa[:, AUX_MQ : AUX_MQ + 256] = mloc0
        for b in range(1, 4):
            auxa[:, AUX_MQ + 256 * b : AUX_MQ + 256 * b + 256] = mloc1
        # strided additive mask: valid iff sidx <= (qs + c) - 46
        ii = qs + np.arange(SQ)[None, :]
        ms = np.where(SIDX[:, None] <= ii - W, 0.0, MASKV).astype(np.float32)
        msadd = np.ascontiguousarray(np.tile(ms, (1, 4))).astype(NP_DT)
        in_maps.append(
            {
                "ta": ta,
                "wq": wq_t,
                "wo": wo_t,
                "aux": auxa.astype(NP_DT),
                "msadd": msadd,
            }
        )
    return in_maps


_NC_CACHE = {}


def get_nc():
    if "nc" not in _NC_CACHE:
        _NC_CACHE["nc"] = build_nc()
    return _NC_CACHE["nc"]


def kernel(x, Wq, Wk, Wv, Wo):
    nc = get_nc()
    in_maps = host_prep(x, Wq, Wk, Wv, Wo)
    res = run_bass_kernel_spmd(nc, in_maps, core_ids=list(range(NCORES)))
    yrows = np.concatenate([r["y"] for r in res.results], axis=0)  # [S, D]
    return np.ascontiguousarray(yrows).reshape(B, S, D).astype(np.float32)


# revision 68
# speedup vs baseline: 1.2416x; 1.2416x over previous
"""CronRootAttention (causal sqrt-N sparse attention + GQA projections) on 8 TRN2 cores.

Sharding: pure sequence shard — each core owns 256 queries, computes all 16
heads for them. Weights replicated; kv projections computed per-core for the
local 320-key span plus the 44 shared strided keys.

v4 dataflow:
  - Local attention in 64-query blocks: each block's 46-wide causal band fits
    one [128-key, 64-query] tile. Blocks are packed in pairs so every phase-B
    PSUM tile is exactly one bank ([128,512] f32) and every mask is an
    additive PE identity-preload (exp then reads masked scores directly).
  - q-projection runs as 4 mini-passes (2 accumulators each) interleaved
    INTO the attention g-loop: mini-pass g+1's matmuls sit between g's score
    and PV sections, covering the exp/normalize engine latencies, and each
    mini-pass is paced by one contiguous wq quarter-chunk DMA.
  - PV head-pairs accumulate into one [65,512] bank; denominators (ones
    column of v) are normalized via one batched reciprocal + gpsimd
    partition-broadcast + two DVE muls per pair.
  - Output projection is one solid PE streak at the end (k-tile-major, so
    the last g's normalize latency hides under k-tiles 0-5), staged out as
    two [128,1024] DMAs.
  - Inputs packed into 5 dram tensors in k-tile-interleaved layout, DMA'd in
    PE-consumption order (transfers are a serial ~332B/ns stream).
"""

import math
import sys

sys.path.insert(0, "/opt/trn_rl_repo")

import numpy as np
import ml_dtypes
import concourse.bass as bass
import concourse.tile as tile
from concourse import bacc, mybir
from concourse.bass_utils import run_bass_kernel_spmd

F32 = mybir.dt.float32
BF16 = mybir.dt.bfloat16
EXP = mybir.ActivationFunctionType.Exp
COPY = mybir.ActivationFunctionType.Copy
MULT = mybir.AluOpType.mult

MM_DT = BF16
NP_DT = ml_dtypes.bfloat16

# Problem constants (hardcoded per contract).
B, S, D = 1, 2048, 1024
H, H_KV, HD = 16, 4, 64
W = int(math.ceil(math.sqrt(S)))  # 46
NCORES = 8
SQ = S // NCORES  # 256 queries per core
KSPAN = 320  # local key span: [qs-64, qs+256)
SIDX = np.arange(W - 1, S, W)  # strided key positions
NS = len(SIDX)  # 44
KT = D // 128  # 8 contraction k-tiles
MASKV = -400.0  # additive mask value; exp(0.125*(qk-400)) == 0

# packed t_a column layout per k-tile: [xkv 320 | xs 44 | wk 256 | wv 260]
TA_XS = KSPAN  # 320
TA_WK = KSPAN + NS  # 364
TA_WV = TA_WK + 256  # 620
TA_W = TA_WV + 260  # 880
# packed aux layout: [ident 128 | mquad 4*256]
AUX_ID, AUX_MQ = 0, 128
AUX_W = 128 + 1024  # 1152


def build_nc():
    nc = bacc.Bacc("TRN2", target_bir_lowering=False, debug=False, num_devices=1)
    ta = nc.dram_tensor("ta", [128, KT, TA_W], MM_DT, kind="ExternalInput").ap()
    # wq chunk-major: [128, 4 col-chunks, KT, 256]; one DMA per mini-pass.
    wq = nc.dram_tensor("wq", [128, 4, KT, 256], MM_DT, kind="ExternalInput").ap()
    wo = nc.dram_tensor("wo", [128, KT, D], MM_DT, kind="ExternalInput").ap()
    aux = nc.dram_tensor("aux", [128, AUX_W], MM_DT, kind="ExternalInput").ap()
    msadd = nc.dram_tensor("msadd", [NS, 1024], MM_DT, kind="ExternalInput").ap()
    y = nc.dram_tensor("y", [SQ, D], MM_DT, kind="ExternalOutput").ap()

    with tile.TileContext(nc) as tc:
        with (
            tc.tile_pool(name="consts", bufs=1) as consts,
            tc.tile_pool(name="work", bufs=1) as work,
        ):
            ta_sb = consts.tile([128, KT, TA_W], MM_DT)
            wq_sb = consts.tile([128, 4, KT, 256], MM_DT)
            wo_sb = consts.tile([128, KT, D], MM_DT)
            aux_sb = consts.tile([128, AUX_W], MM_DT)
            msadd_sb = consts.tile([NS, 1024], MM_DT)

            # DMA stream in PE-consumption order; aux lands before ta[4:8]
            # so the ks/vs matmuls can fill the PE hole while ta[4:8] is in
            # flight.
            nc.sync.dma_start(out=ta_sb[:, 0, 0:620], in_=ta[:, 0, 0:620])
            nc.sync.dma_start(out=ta_sb[:, 0, 620:880], in_=ta[:, 0, 620:880])
            nc.sync.dma_start(out=ta_sb[:, 1:2, :], in_=ta[:, 1:2])
            nc.sync.dma_start(out=ta_sb[:, 2:4, :], in_=ta[:, 2:4])
            nc.sync.dma_start(out=ta_sb[:, 4:6, :], in_=ta[:, 4:6])
            nc.sync.dma_start(out=ta_sb[:, 6:8, :], in_=ta[:, 6:8])
            nc.sync.dma_start(out=aux_sb[:], in_=aux)
            nc.sync.dma_start(out=msadd_sb[:], in_=msadd)
            for c in range(4):
                nc.sync.dma_start(out=wq_sb[:, c], in_=wq[:, c])
            for t in range(2):
                nc.sync.dma_start(out=wo_sb[:, 4 * t : 4 * t + 4, :],
                                  in_=wo[:, 4 * t : 4 * t + 4])

            def xkv(kt):
                return ta_sb[:, kt, 0:KSPAN]

            def wk_sl(kt, ot):
                return ta_sb[:, kt, TA_WK + 128 * ot : TA_WK + 128 * ot + 128]

            def wv_sl(kt):
                return ta_sb[:, kt, TA_WV : TA_WV + 260]

            id_sb = aux_sb[:, AUX_ID : AUX_ID + 128]
            mquad = aux_sb[:, AUX_MQ : AUX_MQ + 1024]

            # work tiles (attention matmul operands all at base partition 0)
            q_sb = work.tile([64, H, SQ], MM_DT)
            k_sb = work.tile([64, 4, KSPAN + NS], MM_DT)  # [k_T 320 | ks_T 44]
            v_sb = work.tile([128, 4, 260], MM_DT)  # 4 stationary alignments
            vs_sb = work.tile([NS, 260], MM_DT)
            attn_sb = work.tile([128, KT, SQ], MM_DT)

            _eng = [0]

            def copy_any(out, in_):
                e = _eng[0] % 2
                _eng[0] += 1
                if e == 0:
                    nc.scalar.activation(out, in_, COPY)
                else:
                    nc.vector.tensor_copy(out, in_)

            # ---- phase A: k+ks and v+vs projections (xs columns are
            # packed right after xkv, so the strided keys ride along in the
            # same accumulators: k moving = [xkv|xs], v stationary includes
            # the xs columns as extra output rows) ----
            with tc.tile_pool(name="psA1", bufs=1, space="PSUM") as psA1:
                kps = [psA1.tile([128, 512], F32, tag="kp", bufs=2, name=f"kp{_}") for _ in range(2)]
                vps = [psA1.tile([128, 512], F32, tag="vp", bufs=3, name=f"vp{_}") for _ in range(3)]
                for kt in range(KT):
                    st, sp = kt == 0, kt == KT - 1
                    for ot in range(2):
                        nc.tensor.matmul(
                            kps[ot][:, 0 : KSPAN + NS], wk_sl(kt, ot),
                            ta_sb[:, kt, 0 : KSPAN + NS],
                            start=st, stop=sp,
                        )
                    for mt in range(2):
                        nc.tensor.matmul(
                            vps[mt][:, 0:260],
                            ta_sb[:, kt, 128 * mt : 128 * mt + 128],
                            wv_sl(kt),
                            start=st, stop=sp,
                        )
                    nc.tensor.matmul(
                        vps[2][0:108, 0:260], ta_sb[:, kt, 256:364], wv_sl(kt),
                        start=st, stop=sp,
                    )
                for ot in range(2):
                    copy_any(k_sb[:, 2 * ot, :], kps[ot][0:64, 0 : KSPAN + NS])
                    copy_any(k_sb[:, 2 * ot + 1, :], kps[ot][64:128, 0 : KSPAN + NS])
                # v alignments: tiles cover span rows [64b, 64b+128)
                copy_any(v_sb[:, 0, :], vps[0][:, 0:260])
                copy_any(v_sb[:, 2, :], vps[1][:, 0:260])
                copy_any(v_sb[0:64, 1, :], vps[0][64:128, 0:260])
                copy_any(v_sb[64:128, 1, :], vps[1][0:64, 0:260])
                copy_any(v_sb[0:64, 3, :], vps[1][64:128, 0:260])
                copy_any(v_sb[64:128, 3, :], vps[2][0:64, 0:260])
                copy_any(vs_sb[:], vps[2][64:108, 0:260])
                for t in range(4):
                    ones_cols = v_sb[:, t, :].rearrange(
                        "p (g c) -> p g c", g=4
                    )[:, :, 64]
                    nc.gpsimd.memset(ones_cols, 1.0)
                vs_ones = vs_sb[:].rearrange("p (g c) -> p g c", g=4)[:, :, 64]
                nc.gpsimd.memset(vs_ones, 1.0)
                # first q mini-pass in a short-lived pool using the 3 banks
                # psA1 doesn't hold — its matmuls run while the projection
                # copies drain, instead of idling PE until the B pools open.
                with tc.tile_pool(name="psq0", bufs=1, space="PSUM") as psq0:
                    qpass0_tiles = [
                        psq0.tile([128, SQ], F32, tag="qp", bufs=2,
                                  name=f"qz{_}")
                        for _ in range(2)
                    ]
                    for kt in range(KT):
                        st, sp = kt == 0, kt == KT - 1
                        for i in range(2):
                            nc.tensor.matmul(
                                qpass0_tiles[i][:],
                                wq_sb[:, 0, kt, 128 * i : 128 * i + 128],
                                xkv(kt)[:, 64:320],
                                start=st, stop=sp,
                            )
                    for i in range(2):
                        copy_any(q_sb[:, 2 * i, :], qpass0_tiles[i][0:64, :])
                        copy_any(q_sb[:, 2 * i + 1, :], qpass0_tiles[i][64:128, :])

            # ---- phase B: attention g-loop with q mini-passes woven in ----
            def q_sl(h, c0, c1):
                return q_sb[:, h, c0:c1]

            def k_sl(g, b):
                return k_sb[:, g, 64 * b : 64 * b + 128]

            def ks_sl(g):
                return k_sb[:, g, KSPAN : KSPAN + NS]

            with (
                tc.tile_pool(name="ps_blk", bufs=1, space="PSUM") as psb,
                tc.tile_pool(name="ps_pv", bufs=1, space="PSUM") as pspv,
                tc.tile_pool(name="ptiles", bufs=1) as pt,
                tc.tile_pool(name="small", bufs=1) as sm,
            ):
                def qpass(c):
                    qps = [
                        psq.tile([128, SQ], F32, tag="qp", bufs=2,
                                 name=f"qp{c}{_}")
                        for _ in range(2)
                    ]
                    for kt in range(KT):
                        st, sp = kt == 0, kt == KT - 1
                        for i in range(2):
                            nc.tensor.matmul(
                                qps[i][:],
                                wq_sb[:, c, kt, 128 * i : 128 * i + 128],
                                xkv(kt)[:, 64:320],
                                start=st, stop=sp,
                            )
                    for i in range(2):
                        ot = 2 * c + i
                        copy_any(q_sb[:, 2 * ot, :], qps[i][0:64, :])
                        copy_any(q_sb[:, 2 * ot + 1, :], qps[i][64:128, :])

                def scores_g(g):
                    # strided: 2 head-pair tiles [44,512]; mask via identity
                    # preload, so exp output is final. Exp results land in
                    # the per-g halves of merged pstr/pbq SBUF tiles.
                    pstr = pt.tile([NS, 1024], MM_DT, tag="pstr", bufs=3)
                    pbq = pt.tile([128, 1024], MM_DT, tag="pb", bufs=3)
                    for u in range(2):
                        stt = psb.tile([NS, 512], F32, tag="blk", bufs=4,
                                       name=f"st{u}")
                        nc.tensor.matmul(
                            stt[:], id_sb[0:NS, 0:NS],
                            msadd_sb[:, 512 * u : 512 * u + 512],
                            start=True, stop=False, skip_group_check=True,
                        )
                        for i in range(2):
                            nc.tensor.matmul(
                                stt[:, 256 * i : 256 * i + 256],
                                ks_sl(g),
                                q_sl(4 * g + 2 * u + i, 0, SQ),
                                start=False, stop=True,
                                skip_group_check=True,
                            )
                        nc.scalar.activation(
                            pstr[:, 512 * u : 512 * u + 512], stt[:], EXP,
                            scale=0.125,
                        )
                    # local: block pairs packed into one bank [128, 512]
                    for half in range(2):
                        spp = psb.tile([128, 512], F32, tag="blk", bufs=4,
                                       name=f"sp{half}")
                        nc.tensor.matmul(
                            spp[:], id_sb,
                            mquad[:, 512 * half : 512 * half + 512],
                            start=True, stop=False, skip_group_check=True,
                        )
                        for j in range(2):
                            b = 2 * half + j
                            for hh in range(4):
                                nc.tensor.matmul(
                                    spp[:, 256 * j + 64 * hh : 256 * j + 64 * hh + 64],
                                    k_sl(g, b),
                                    q_sl(4 * g + hh, 64 * b, 64 * b + 64),
                                    start=False, stop=(j == 1 and hh == 3),
                                    skip_group_check=True,
                                )
                        nc.scalar.activation(
                            pbq[:, 512 * half : 512 * half + 512], spp[:], EXP,
                            scale=0.125,
                        )
                    return pstr, pbq

                def pv_g(g, pstr, pbq):
                    # PV per head pair into one [65,512] bank + normalize.
                    # g3 runs pair 1 (k-tile 7) first so phase C's last
                    # k-tiles unblock in emission order.
                    for u in (0, 1) if g < 3 else (1, 0):
                        pvp = pspv.tile([65, 512], F32, tag="pv", bufs=2)
                        for i in range(2):
                            hh = 2 * u + i
                            c = 256 * i
                            nc.tensor.matmul(
                                pvp[:, c : c + 256],
                                vs_sb[:, 65 * g : 65 * g + 65],
                                pstr[:, 256 * hh : 256 * hh + 256],
                                start=True, stop=False,
                                skip_group_check=True,
                            )
                            for b in range(4):
                                nc.tensor.matmul(
                                    pvp[:, c + 64 * b : c + 64 * b + 64],
                                    v_sb[:, b, 65 * g : 65 * g + 65],
                                    pbq[:, 256 * b + 64 * hh : 256 * b + 64 * hh + 64],
                                    start=False, stop=b == 3,
                                    skip_group_check=True,
                                )
                        rt = sm.tile([1, 512], F32, tag="rt", bufs=3)
                        nc.vector.reciprocal(rt[:], pvp[64:65, :])
                        rep = sm.tile([64, 512], F32, tag="rep", bufs=3)
                        nc.gpsimd.partition_broadcast(rep[:], rt[:], channels=64)
                        for i in range(2):
                            h = 4 * g + 2 * u + i
                            nc.vector.tensor_tensor(
                                out=attn_sb[64 * (h % 2) : 64 * (h % 2) + 64, h // 2, :],
                                in0=pvp[0:64, 256 * i : 256 * i + 256],
                                in1=rep[:, 256 * i : 256 * i + 256],
                                op=MULT,
                            )

                with tc.tile_pool(name="psq", bufs=1, space="PSUM") as psq:
                    for g in range(4):
                        sc = scores_g(g)
                        if g < 3:
                            qpass(g + 1)
                        pv_g(g, *sc)

                # ---- phase C: output projection. The 4 y accumulators take
                # the blk ring's 4 banks (no pool-close barrier separates
                # it from phase B); k-tile-major so the last g's normalize
                # latency hides under k-tiles 0-5.
                with tc.tile_pool(name="yout", bufs=2) as yo:
                    CHUNKS = [(0, 0), (0, 512), (1, 0), (1, 512)]
                    yts = [psb.tile([128, 512], F32, tag="blk", bufs=4,
                                    name=f"yt{i}") for i in range(4)]
                    for kt in range(KT - 2):
                        for i in range(4):
                            qt, c0 = CHUNKS[i]
                            nc.tensor.matmul(
                                yts[i][:],
                                attn_sb[:, kt, 128 * qt : 128 * qt + 128],
                                wo_sb[:, kt, c0 : c0 + 512],
                                start=kt == 0,
                                stop=False,
                            )
                    # k-tiles 7 then 6 (kt7's heads normalize first); qt1's
                    # chunks finish both k-tiles before qt0's start, so its
                    # staging chain overlaps qt0's final matmuls.
                    for group in ((2, 3), (0, 1)):
                        for kt in (7, 6):
                            for i in group:
                                qt, c0 = CHUNKS[i]
                                nc.tensor.matmul(
                                    yts[i][:],
                                    attn_sb[:, kt, 128 * qt : 128 * qt + 128],
                                    wo_sb[:, kt, c0 : c0 + 512],
                                    start=False,
                                    stop=kt == 6,
                                    skip_group_check=True,
                                )
                    for qt in (1, 0):
                        ysb = yo.tile([128, D], MM_DT, tag="ysb", name=f"ysb{qt}")
                        nc.scalar.activation(ysb[:, 0:512], yts[2 * qt][:], COPY)
                        nc.vector.tensor_copy(ysb[:, 512:1024], yts[2 * qt + 1][:])
                        nc.sync.dma_start(
                            out=y[128 * qt : 128 * qt + 128, :], in_=ysb[:]
                        )
    nc.compile()
    return nc


def host_prep(x, Wq, Wk, Wv, Wo):
    """Build per-core input maps (pure data reordering, no FLOPs)."""
    x2 = np.asarray(x, np.float32).reshape(S, D)
    xT = np.ascontiguousarray(x2.T)  # [D, S]
    xpad = np.zeros((D, 64 + S), np.float32)
    xpad[:, 64:] = xT
    xs = xT[:, SIDX]  # [D, 44]
    wkT = np.asarray(Wk, np.float32).T  # [D, 256]
    wvT = np.asarray(Wv, np.float32).T  # [D, 256]
    wv = np.zeros((D, 260), np.float32)
    for g in range(4):
        wv[:, 65 * g : 65 * g + 64] = wvT[:, 64 * g : 64 * g + 64]
    # wq chunk-major: [128, 4, KT, 256]; chunk c = output channels 256c..
    wq_t = np.ascontiguousarray(
        np.asarray(Wq, np.float32).T.reshape(KT, 128, 4, 256).transpose(1, 2, 0, 3)
    ).astype(NP_DT)
    wo_t = np.ascontiguousarray(
        np.asarray(Wo, np.float32).T.reshape(KT, 128, D).transpose(1, 0, 2)
    ).astype(NP_DT)

    # local band mask: i = qb + c, j = qb - 64 + r -> valid iff
    # 0 <= c + 64 - r <= 45 (identical for every block except block 0 of
    # core 0, whose keys j<0 must also be killed).
    r = np.arange(128)[:, None]
    c = np.arange(64)[None, :]
    band = (c + 64 - r >= 0) & (c + 64 - r <= 45)
    mloc1 = np.tile(np.where(band, 0.0, MASKV).astype(np.float32), (1, 4))

    ident = np.eye(128, dtype=np.float32)

    in_maps = []
    for core in range(NCORES):
        qs = SQ * core
        xkv = xpad[:, qs : qs + KSPAN]  # [D, 320]
        ta = np.concatenate(
            [
                xkv.reshape(KT, 128, KSPAN),
                xs.reshape(KT, 128, NS),
                wkT.reshape(KT, 128, 256),
                wv.reshape(KT, 128, 260),
            ],
            axis=2,
        ).transpose(1, 0, 2)
        ta = np.ascontiguousarray(ta).astype(NP_DT)
        jglob = qs - 64 + np.arange(128)[:, None]
        band0 = band & (jglob >= 0)
        mloc0 = np.tile(np.where(band0, 0.0, MASKV).astype(np.float32), (1, 4))
        auxa = np.zeros((128, AUX_W), np.float32)
        auxa[:, AUX_ID : AUX_ID + 128] = ident
        # mquad: [block0 mask | block1-3 mask x3]
        auxa[:, AUX_MQ : AUX_MQ + 256] = mloc0
        for b in range(1, 4):
            auxa[:, AUX_MQ + 256 * b : AUX_MQ + 256 * b + 256] = mloc1
        # strided additive mask: valid iff sidx <= (qs + c) - 46
        ii = qs + np.arange(SQ)[None, :]
        ms = np.where(SIDX[:, None] <= ii - W, 0.0, MASKV).astype(np.float32)
        msadd = np.ascontiguousarray(np.tile(ms, (1, 4))).astype(NP_DT)
        in_maps.append(
            {
                "ta": ta,
                "wq": wq_t,
                "wo": wo_t,
                "aux": auxa.astype(NP_DT),
                "msadd": msadd,
            }
        )
    return in_maps


_NC_CACHE = {}


def get_nc():
    if "nc" not in _NC_CACHE:
        _NC_CACHE["nc"] = build_nc()
    return _NC_CACHE["nc"]


def kernel(x, Wq, Wk, Wv, Wo):
    nc = get_nc()
    in_maps = host_prep(x, Wq, Wk, Wv, Wo)
    res = run_bass_kernel_spmd(nc, in_maps, core_ids=list(range(NCORES)))
    yrows = np.concatenate([r["y"] for r in res.results], axis=0)  # [S, D]
    return np.ascontiguousarray(yrows).reshape(B, S, D).astype(np.float32)
